# revision 11
# baseline (speedup 1.0000x reference)
"""MoE router GEMM on 8 TRN2 NeuronCores.

logits[t, e] = sum_d x[t, d] * w[e, d]
  x: [16384, 6144] bf16, w: [768, 6144] bf16, out fp32 [16384, 768].

Sharding: tokens split 8 ways (data parallel), weight replicated; each core
computes its [2048, 768] logits shard; host concatenates (the "all-gather").

Per-core kernel: w.T staged once into SBUF as [128 ki, 48 ko, 768 e] via XBAR
DMA-transpose (streaming matmul operand, resident); x staged in [128 ki, 48 ko,
M_TILE t] XBAR-transposed tiles (stationary operand); PSUM accumulates over the
48 k-subtiles for each 128-token row block; DVE evicts PSUM->SBUF; DMA out.
"""

import sys

for _p in ("/opt/trn_rl_repo", "/root/.axon_site/_ro/trn_rl_repo"):
    if _p not in sys.path:
        sys.path.insert(0, _p)

import numpy as np

N_CORES = 8
T_FULL = 16384
T = T_FULL // N_CORES  # 2048 tokens per core
D = 6144
E = 768
P = 128
KO = D // P  # 48 k-subtiles

_NC_CACHE = {}

# v3 hybrid: ko8 k-subtiles (ko8*128 of the 6144 contraction dim) run as
# fp8-e4m3 DoubleRow matmul pairs (2 k-subtiles per PE pass), the rest stays
# bf16. The bf16 weight half is pre-scaled by W_SCALE=64 on the host so both
# parts accumulate 64*logits in one PSUM group (w*64 keeps fp8 w in e4m3's
# normal range); the final PSUM->SBUF eviction multiplies by 1/64.
W_SCALE = 64.0


def _build_nc(
    reps=1,
    m_tile=512,
    xbufs=2,
    obufs=3,
    psum_bufs=2,
    n_split=512,
    style="v1",
    ko8=12,
    mini=False,
):
    import concourse.bacc as bacc
    import concourse.mybir as mybir
    import concourse.tile as tile

    nc = bacc.Bacc("TRN2", target_bir_lowering=False, debug=False, num_devices=N_CORES)

    if style == "v3":
        with_tc = tile.TileContext(nc)
        with with_tc as tc:
            _build_v3(nc, tc, tile, mybir, reps=reps, ko8=ko8, m_tile=m_tile,
                      xbufs=xbufs, obufs=obufs, psum_bufs=psum_bufs, mini=mini)
        nc.compile()
        return nc

    x = nc.dram_tensor("hidden_states", [T, D], mybir.dt.bfloat16, kind="ExternalInput")
    w = nc.dram_tensor("weight", [E, D], mybir.dt.bfloat16, kind="ExternalInput")
    out = nc.dram_tensor("out", [T, E], mybir.dt.float32, kind="ExternalOutput")

    with tile.TileContext(nc) as tc:
        if reps == 0:
            # null kernel: one tiny DMA roundtrip, for launch-overhead measurement
            with tc.tile_pool(name="null", bufs=1) as pool:
                t_in = pool.tile([P, 256], mybir.dt.bfloat16)
                nc.sync.dma_start(t_in[:], w[0:P, 0:256])
                t_out = pool.tile([P, 256], mybir.dt.float32)
                nc.vector.tensor_copy(t_out[:], t_in[:])
                nc.sync.dma_start(out[0:P, 0:256], t_out[:])
            nc.compile()
            return nc
        if style == "v2":
            _build_v2(nc, tc, tile, mybir, x, w, out, reps=reps, xbufs=xbufs,
                      obufs=obufs, psum_bufs=psum_bufs)
            nc.compile()
            return nc
        with (
            tc.tile_pool(name="wpool", bufs=1) as wpool,
            tc.tile_pool(name="xpool", bufs=xbufs) as xpool,
            tc.tile_pool(name="opool", bufs=obufs) as opool,
            tc.tile_pool(name="psum", bufs=psum_bufs, space="PSUM") as psum_pool,
        ):
            # w.T resident: wt[ki, ko, e] = w[e, ko*128 + ki]
            if style == "v1w2":
                # two expert halves so the PE can start after half 0 lands
                cw = E // 2
                wt0 = wpool.tile([P, KO, cw], mybir.dt.bfloat16, name="wt0")
                wt1 = wpool.tile([P, KO, cw], mybir.dt.bfloat16, name="wt1")
                nc.sync.dma_start_transpose(
                    wt0[:], w[0:cw].rearrange("e (ko ki) -> e ko ki", ki=P)
                )
                nc.sync.dma_start_transpose(
                    wt1[:], w[cw:E].rearrange("e (ko ki) -> e ko ki", ki=P)
                )
                wparts = [(0, cw, wt0), (cw, cw, wt1)]
            else:
                wt = wpool.tile([P, KO, E], mybir.dt.bfloat16)
                nc.sync.dma_start_transpose(
                    wt[:], w.rearrange("e (ko ki) -> e ko ki", ki=P)
                )
                wparts = None

            xv = x.rearrange("t (ko ki) -> t ko ki", ki=P)
            for rep in range(reps):
                for mt in range(T // m_tile):
                    # xt[ki, ko, t] = x[mt*m_tile + t, ko*128 + ki]
                    xt = xpool.tile([P, KO, m_tile], mybir.dt.bfloat16, tag="xt")
                    nc.sync.dma_start_transpose(
                        xt[:], xv[mt * m_tile : (mt + 1) * m_tile]
                    )
                    for ms in range(m_tile // P):
                        ptile = psum_pool.tile([P, E], mybir.dt.float32, tag="ps")
                        ot = opool.tile([P, E], mybir.dt.float32, tag="ot")
                        lhs = xt[:, :, ms * P : (ms + 1) * P]
                        if wparts is not None:
                            for n0, cw_, wtile in wparts:
                                for ks in range(KO):
                                    nc.tensor.matmul(
                                        ptile[:, n0 : n0 + cw_],
                                        lhs[:, ks],
                                        wtile[:, ks],
                                        start=(ks == 0),
                                        stop=(ks == KO - 1),
                                    )
                        else:
                            for n0 in range(0, E, n_split):
                                n1 = min(n0 + n_split, E)
                                for ks in range(KO):
                                    nc.tensor.matmul(
                                        ptile[:, n0:n1],
                                        lhs[:, ks],
                                        wt[:, ks, n0:n1],
                                        start=(ks == 0),
                                        stop=(ks == KO - 1),
                                    )
                        nc.vector.tensor_copy(ot[:], ptile[:])
                        r0 = mt * m_tile + ms * P
                        nc.sync.dma_start(out[r0 : r0 + P, :], ot[:])

    nc.compile()
    return nc


def _build_v2(nc, tc, tile, mybir, x, w, out, reps=1, xbufs=6, obufs=3, psum_bufs=2):
    """Head-latency-optimized layout: w in two 384-expert halves; 128-token x
    tiles; ms-pairs processed chunk-major so the PE starts on w-half 0 while
    half 1 is still streaming in."""
    NCH = 2
    CW = E // NCH  # 384
    MT = P  # 128 tokens per x tile
    with (
        tc.tile_pool(name="wpool", bufs=1) as wpool,
        tc.tile_pool(name="xpool", bufs=xbufs) as xpool,
        tc.tile_pool(name="opool", bufs=obufs) as opool,
        tc.tile_pool(name="psum", bufs=psum_bufs, space="PSUM") as psum_pool,
    ):
        xv = x.rearrange("t (ko ki) -> t ko ki", ki=P)
        # wt_c[ki, ko, e] = w[c*CW + e, ko*128 + ki]
        wts = []
        for c in range(NCH):
            wt = wpool.tile([P, KO, CW], mybir.dt.bfloat16, name=f"wt{c}")
            wts.append(wt)
        # emission order: w half 0 first, then the first x pair, then w half 1
        nc.sync.dma_start_transpose(
            wts[0][:], w[0:CW].rearrange("e (ko ki) -> e ko ki", ki=P)
        )
        first_pair_xt = []
        for j in range(2):
            xt = xpool.tile([P, KO, MT], mybir.dt.bfloat16, tag="xt", name=f"xt_h{j}")
            nc.sync.dma_start_transpose(xt[:], xv[j * MT : (j + 1) * MT])
            first_pair_xt.append(xt)
        nc.sync.dma_start_transpose(
            wts[1][:], w[CW:E].rearrange("e (ko ki) -> e ko ki", ki=P)
        )

        n_mt = T // MT  # 16
        for rep in range(reps):
            for pair in range(n_mt // 2):
                xts = []
                for j in range(2):
                    mt = 2 * pair + j
                    if rep == 0 and pair == 0:
                        xt = first_pair_xt[j]
                    else:
                        xt = xpool.tile(
                            [P, KO, MT], mybir.dt.bfloat16, tag="xt", name=f"xt{mt}"
                        )
                        nc.sync.dma_start_transpose(
                            xt[:], xv[mt * MT : (mt + 1) * MT]
                        )
                    xts.append(xt)
                ptiles = [
                    psum_pool.tile([P, E], mybir.dt.float32, tag="ps", name=f"ps{j}")
                    for j in range(2)
                ]
                for c in range(NCH):
                    for j in range(2):
                        for ks in range(KO):
                            nc.tensor.matmul(
                                ptiles[j][:, c * CW : (c + 1) * CW],
                                xts[j][:, ks],
                                wts[c][:, ks],
                                start=(ks == 0),
                                stop=(ks == KO - 1),
                            )
                for j in range(2):
                    mt = 2 * pair + j
                    ot = opool.tile([P, E], mybir.dt.float32, tag="ot", name=f"ot{mt}")
                    nc.vector.tensor_copy(ot[:], ptiles[j][:])
                    r0 = mt * MT
                    nc.scalar.dma_start(out[r0 : r0 + MT, :], ot[:])


def _build_v3(nc, tc, tile, mybir, reps, ko8, m_tile, xbufs, obufs, psum_bufs,
              mini=False):
    """bf16 + fp8-DoubleRow hybrid.

    Contraction split: first KOB=48-ko8 k-subtiles in bf16 (w pre-scaled x64),
    last ko8 subtiles in e4m3 DoubleRow (w8 = e4m3(64*w), x8 = e4m3(x)), all
    accumulating 64*logits into one PSUM group per 256-expert chunk. fp8
    operands arrive pre-transposed from the host ([ki, ko, t/e] layout) so only
    the bf16 x path needs on-device DMA transpose.
    """
    KOB = KO - ko8
    DBF = KOB * P
    assert ko8 % 2 == 0 and ko8 >= 0
    f8 = mybir.dt.float8e4

    x_bf = nc.dram_tensor("x_bf", [T, DBF], mybir.dt.bfloat16, kind="ExternalInput")
    x_f8t = nc.dram_tensor("x_f8t", [P, ko8, T], f8, kind="ExternalInput")
    w_bf = nc.dram_tensor("w_bf", [E, DBF], mybir.dt.bfloat16, kind="ExternalInput")
    w_f8t = nc.dram_tensor("w_f8t", [P, ko8, E], f8, kind="ExternalInput")
    out = nc.dram_tensor("out", [T, E], mybir.dt.float32, kind="ExternalOutput")

    CH = 256
    NCH = E // CH
    DR = mybir.MatmulPerfMode.DoubleRow

    with (
        tc.tile_pool(name="wpool", bufs=1) as wpool,
        tc.tile_pool(name="xpool", bufs=xbufs) as xpool,
        tc.tile_pool(name="x8pool", bufs=xbufs) as x8pool,
        tc.tile_pool(name="opool", bufs=obufs) as opool,
        tc.tile_pool(name="psum", bufs=psum_bufs, space="PSUM") as psum_pool,
    ):
        wt = wpool.tile([P, KOB, E], mybir.dt.bfloat16, name="wt")
        nc.sync.dma_start_transpose(
            wt[:], w_bf.rearrange("e (ko ki) -> e ko ki", ki=P)
        )
        wt8 = wpool.tile([P, ko8, E], f8, name="wt8")
        nc.sync.dma_start(wt8[:], w_f8t[:])

        xv = x_bf.rearrange("t (ko ki) -> t ko ki", ki=P)
        n_mt = 1 if mini else T // m_tile
        for rep in range(reps):
            for mt in range(n_mt):
                xt = xpool.tile([P, KOB, m_tile], mybir.dt.bfloat16, tag="xt")
                nc.sync.dma_start_transpose(
                    xt[:], xv[mt * m_tile : (mt + 1) * m_tile]
                )
                xt8 = x8pool.tile([P, ko8, m_tile], f8, tag="xt8")
                nc.sync.dma_start(
                    xt8[:], x_f8t[:, :, mt * m_tile : (mt + 1) * m_tile]
                )
                for ms in range(1 if mini else m_tile // P):
                    ptile = psum_pool.tile([P, E], mybir.dt.float32, tag="ps")
                    ot = opool.tile([P, E], mybir.dt.float32, tag="ot")
                    lhs = xt[:, :, ms * P : (ms + 1) * P]
                    lhs8 = xt8[:, :, ms * P : (ms + 1) * P]
                    for c in range(NCH):
                        n0 = c * CH
                        n1 = n0 + CH
                        for ks in range(KOB):
                            nc.tensor.matmul(
                                ptile[:, n0:n1],
                                lhs[:, ks],
                                wt[:, ks, n0:n1],
                                start=(ks == 0),
                                stop=(ks == KOB - 1) and ko8 == 0,
                            )
                        for p8 in range(ko8 // 2):
                            nc.tensor.matmul(
                                ptile[:, n0:n1],
                                lhs8[:, 2 * p8 : 2 * p8 + 2],
                                wt8[:, 2 * p8 : 2 * p8 + 2, n0:n1],
                                start=False,
                                stop=(p8 == ko8 // 2 - 1),
                                perf_mode=DR,
                            )
                    nc.vector.tensor_scalar_mul(ot[:], ptile[:], 1.0 / W_SCALE)
                    r0 = mt * m_tile + ms * P
                    nc.sync.dma_start(out[r0 : r0 + P, :], ot[:])


def _get_nc(**kw):
    key = tuple(sorted(kw.items()))
    if key not in _NC_CACHE:
        _NC_CACHE[key] = _build_nc(**kw)
    return _NC_CACHE[key]


def _to_bf16_shards(hidden_states, weight):
    import ml_dtypes

    x = np.asarray(hidden_states)
    w = np.asarray(weight)
    if x.dtype != ml_dtypes.bfloat16:
        x = x.astype(ml_dtypes.bfloat16)
    if w.dtype != ml_dtypes.bfloat16:
        w = w.astype(ml_dtypes.bfloat16)
    assert x.shape == (T_FULL, D) and w.shape == (E, D)
    return [
        {"hidden_states": np.ascontiguousarray(x[i * T : (i + 1) * T]), "weight": w}
        for i in range(N_CORES)
    ]


def _to_v3_shards(hidden_states, weight, ko8):
    import ml_dtypes

    x = np.asarray(hidden_states)
    w = np.asarray(weight)
    if x.dtype != ml_dtypes.bfloat16:
        x = x.astype(ml_dtypes.bfloat16)
    assert x.shape == (T_FULL, D) and w.shape == (E, D)
    DBF = D - ko8 * P
    x_bf = x[:, :DBF]
    x8 = x[:, DBF:].astype(ml_dtypes.float8_e4m3)
    wf = w.astype(np.float32) * W_SCALE
    w_bf = np.ascontiguousarray(wf[:, :DBF].astype(ml_dtypes.bfloat16))
    w8 = wf[:, DBF:].astype(ml_dtypes.float8_e4m3)
    # [ki, ko, e] layout: w_f8t[ki, ko, e] = w8[e, ko*128 + ki]
    w_f8t = np.ascontiguousarray(w8.reshape(E, ko8, P).transpose(2, 1, 0))
    shards = []
    for i in range(N_CORES):
        xs8 = x8[i * T : (i + 1) * T]
        x_f8t = np.ascontiguousarray(xs8.reshape(T, ko8, P).transpose(2, 1, 0))
        shards.append(
            {
                "x_bf": np.ascontiguousarray(x_bf[i * T : (i + 1) * T]),
                "x_f8t": x_f8t,
                "w_bf": w_bf,
                "w_f8t": w_f8t,
            }
        )
    return shards


def run_sharded(hidden_states, weight, trace=False, **build_kw):
    """Returns (out [16384, 768] fp32, BassKernelResults)."""
    from concourse.bass_utils import run_bass_kernel_spmd

    nc = _get_nc(**build_kw)
    if build_kw.get("style") == "v3":
        in_maps = _to_v3_shards(hidden_states, weight, build_kw.get("ko8", 12))
    else:
        in_maps = _to_bf16_shards(hidden_states, weight)
    res = run_bass_kernel_spmd(nc, in_maps, core_ids=list(range(N_CORES)), trace=trace)
    out = np.concatenate(
        [res.results[i]["out"] for i in range(N_CORES)], axis=0
    ).astype(np.float32, copy=False)
    return out, res


# build_kw used by kernel() and by bench's default config — a single cache
# key so the NEFF is compiled once per process.
DEFAULT_KW = dict(reps=1)


def kernel(hidden_states, weight):
    out, _ = run_sharded(hidden_states, weight, trace=False, **DEFAULT_KW)
    return out

